# revision 27
# baseline (speedup 1.0000x reference)
"""GraphSAGE(2-layer, mean-agg) + KAN MLP + classifier on 8 TRN2 NeuronCores.

v2 design: dst-sharded nodes; A/B table halves split by source shard group
(shards 0-3 vs 4-7) so both layers share one destination permutation
(degree-sorted, stable => pad nodes land at the permutation tail). Tables are
computed SHARDED (one rank contribution = 50 chunks: 49 permA/natural chunks
+ 1 explicit zero chunk for gather padding) and replicated via AllGather,
replacing the replicated full-table GEMMs of v1. Gathers use
single_packet=True to amortize per-packet SDMA overhead on 256B rows.
KAN/classifier run in bf16. Bias terms are folded into the root GEMM via a
rank-1 ones @ bias_row matmul.
"""

import math
import numpy as np
import ml_dtypes

IN_DIM, HID, KAN_Q = 256, 128, 4
P = 128
_F32 = np.float32
_BF16 = ml_dtypes.bfloat16
SINGLE_PACKET = True
SPLIT_NI = 1024          # max indices per dma_gather: 64 descs/engine packets


class CFG:
    def __init__(self, n=50000, e=800000, ncores=8):
        self.N, self.E, self.NCORES = n, e, ncores
        self.SHARD_REAL = n // ncores                  # 6250
        self.NCH = math.ceil(self.SHARD_REAL / P)      # 49
        self.SH = self.NCH * P                         # 6272
        self.NCHC = self.NCH + 1                       # 50 (with zero chunk)
        self.SHC = self.NCHC * P                       # 6400
        self.V2 = ncores * self.SHC                    # 51200
        self.TH = (ncores // 2) * self.SHC             # 25600
        self.PAD_A = self.SH                           # rank0 zero chunk row
        self.PAD_B = (ncores - 1) * self.SHC + self.SH - self.TH  # 25472


# ================================================================ host prep
def _common_calls(deg_sorted_list, cfg):
    """Common per-k chunk counts across cores: nc_k = max_c ceil(#deg>k /128)."""
    degmax = max((int(d[0]) if len(d) else 0) for d in deg_sorted_list)
    calls = []
    for k in range(degmax):
        m = max(int(np.count_nonzero(d > k)) for d in deg_sorted_list)
        calls.append(max(1, math.ceil(m / P)))
    return calls


def _stage_blocks(cols):
    """cols: list of int32 arrays (len multiple of 16) -> [128, S] int16."""
    if not cols:
        return np.zeros((P, 0), np.int16)
    blocks = []
    for col in cols:
        assert col.max(initial=0) < 32768 and col.min(initial=0) >= 0
        blk = col.reshape(-1, 16).T.astype(np.int16)
        blocks.append(np.tile(blk, (8, 1)))
    return np.concatenate(blocks, axis=1)


def _half_idx(src_idx, dst_pos, calls, n_pos, pad_idx):
    degmax = len(calls)
    order = np.argsort(dst_pos, kind="stable")
    sp, si = dst_pos[order], src_idx[order]
    starts = np.zeros(n_pos + 1, np.int64)
    np.cumsum(np.bincount(sp, minlength=n_pos), out=starts[1:])
    rank = np.arange(len(sp)) - starts[sp]
    mat = np.full((n_pos, max(degmax, 1)), pad_idx, np.int32)
    if len(sp):
        mat[sp, rank.astype(np.int64)] = si
    return [mat[:nc * P, k].astype(np.int32) for k, nc in enumerate(calls)]


def preprocess(edge_index, cfg):
    src = np.asarray(edge_index[0], np.int64)
    dst = np.asarray(edge_index[1], np.int64)
    NC, SR, SH, SHC, TH = (cfg.NCORES, cfg.SHARD_REAL, cfg.SH, cfg.SHC, cfg.TH)
    deg = np.bincount(dst, minlength=cfg.N).astype(np.int64)
    src_core = src // SR
    src_local = src - src_core * SR
    halfB = src_core >= NC // 2

    metas = []
    for c in range(NC):
        m = (dst >= c * SR) & (dst < (c + 1) * SR)
        d_loc = dst[m] - c * SR
        hB = halfB[m]
        degA = np.bincount(d_loc[~hB], minlength=SH)
        degB = np.bincount(d_loc[hB], minlength=SH)
        permA = np.argsort(-degA, kind="stable")
        permB = np.argsort(-degB, kind="stable")
        posA = np.empty(SH, np.int64); posA[permA] = np.arange(SH)
        posB = np.empty(SH, np.int64); posB[permB] = np.arange(SH)
        metas.append(dict(m=m, d=d_loc, hB=hB, degA=degA, degB=degB,
                          permA=permA, permB=permB, posA=posA, posB=posB))

    callsA = _common_calls([np.sort(mm["degA"])[::-1] for mm in metas], cfg)
    callsB = _common_calls([np.sort(mm["degB"])[::-1] for mm in metas], cfg)

    posA_all = np.empty(cfg.N, np.int64)
    for c in range(NC):
        posA_all[c * SR:(c + 1) * SR] = metas[c]["posA"][:SR]
    rows1 = src_core * SHC + src_local
    rows2 = src_core * SHC + posA_all[src]

    cores = []
    for c, mm in enumerate(metas):
        hB, msk = mm["hB"], mm["m"]
        dposA = mm["posA"][mm["d"][~hB]]
        dposB = mm["posB"][mm["d"][hB]]

        def build_idx(rows):
            r = rows[msk]
            colsA = _half_idx(r[~hB], dposA, callsA, SH, cfg.PAD_A)
            colsB = _half_idx(r[hB] - TH, dposB, callsB, SH, cfg.PAD_B)
            return _stage_blocks(colsA + colsB)

        deg_l = np.zeros(SH, np.int64)
        deg_l[:SR] = deg[c * SR:(c + 1) * SR]
        inv = (1.0 / np.maximum(deg_l, 1.0)).astype(_F32)
        cores.append(dict(
            idx1=build_idx(rows1), idx2=build_idx(rows2),
            mg=_stage_blocks([mm["posB"][mm["permA"]].astype(np.int32)]),
            inv=np.ascontiguousarray(inv[mm["permA"]].reshape(cfg.NCH, P).T),
            permA=mm["permA"],
        ))
    plans = dict(callsA=callsA, callsB=callsB, sidx=cores[0]["idx1"].shape[1])
    return cores, plans


# ================================================================ device build
def build(cfg, plans, gelu_func="Gelu_apprx_tanh"):
    import contextlib
    import concourse.bacc as bacc
    import concourse.mybir as mybir
    from concourse.tile import TileContext
    from concourse.library_config import mlp

    f32, bf16, i16 = mybir.dt.float32, mybir.dt.bfloat16, mybir.dt.int16
    AF = mybir.ActivationFunctionType
    ALU = mybir.AluOpType
    GELU = getattr(AF, gelu_func)
    NCH, SH, SHC, NCHC, TH, V2 = (cfg.NCH, cfg.SH, cfg.SHC, cfg.NCHC,
                                  cfg.TH, cfg.V2)
    callsA, callsB = plans["callsA"], plans["callsB"]
    SIDX = plans["sidx"]
    SIDXP = max(SIDX, 8)

    nc = bacc.Bacc(None, target_bir_lowering=False, debug=False,
                   num_swdge_queues=4)

    def din(name, shape, dt):
        return nc.dram_tensor(name, shape, dt, kind="ExternalInput")

    xTs = din("xTs", [IN_DIM, SHC], bf16)
    xTpA = din("xTpA", [IN_DIM, SH], bf16)
    Wl1 = din("Wl1", [IN_DIM, HID], bf16)
    Wr1 = din("Wr1", [IN_DIM, HID], bf16)
    Wl2 = din("Wl2", [HID, HID], bf16)
    Wr2 = din("Wr2", [HID, HID], bf16)
    Wk1 = din("Wk1", [HID, HID * KAN_Q], bf16)
    Wk2 = din("Wk2", [HID * KAN_Q, HID], bf16)
    Wc = din("Wc", [HID, 1], bf16)
    brl1 = din("brl1", [1, HID], bf16)
    brl2 = din("brl2", [1, HID], bf16)
    bk1 = din("bk1", [P, KAN_Q], f32)
    bk2 = din("bk2", [P, 1], f32)
    bc = din("bc", [1, 1], f32)
    ident = din("ident", [P, P], bf16)
    idx1 = din("idx1", [P, SIDXP], i16)
    idx2 = din("idx2", [P, SIDXP], i16)
    mg = din("mg", [P, SH // 16], i16)
    inv = din("inv", [P, NCH], f32)
    logits = nc.dram_tensor("logits", [SH], f32, kind="ExternalOutput")

    with TileContext(nc) as tc, contextlib.ExitStack() as ctx:
        dram = ctx.enter_context(tc.tile_pool(name="dram", bufs=1, space="DRAM"))
        const = ctx.enter_context(tc.tile_pool(name="const", bufs=1))
        gp = ctx.enter_context(tc.tile_pool(name="gath", bufs=1))
        accp = ctx.enter_context(tc.tile_pool(name="accs", bufs=1))
        work = ctx.enter_context(tc.tile_pool(name="work", bufs=2))
        wrk1 = ctx.enter_context(tc.tile_pool(name="wrk1", bufs=1))
        psum = ctx.enter_context(tc.tile_pool(name="ps", bufs=2, space="PSUM"))
        psum5 = ctx.enter_context(tc.tile_pool(name="ps5", bufs=2, space="PSUM"))

        table1loc = dram.tile([SHC, HID], bf16)
        table2loc = dram.tile([SHC, HID], bf16)
        table1 = dram.tile([V2, HID], bf16, addr_space="Shared")
        table2 = dram.tile([V2, HID], bf16, addr_space="Shared")
        h1loc = dram.tile([SH, HID], bf16)
        scratchB = dram.tile([SH, HID], f32)

        nc.gpsimd.load_library(mlp)

        def ld(dr, shape, dt, tag, pool=const):
            t = pool.tile(shape, dt, tag=tag)
            nc.sync.dma_start(out=t[:], in_=dr[:])
            return t

        wl1_t = [ld(Wl1[k * P:(k + 1) * P, :], [P, HID], bf16, f"wl1{k}") for k in range(2)]
        wr1_t = [ld(Wr1[k * P:(k + 1) * P, :], [P, HID], bf16, f"wr1{k}") for k in range(2)]
        wl2_t = ld(Wl2, [HID, HID], bf16, "wl2")
        wr2_t = ld(Wr2, [HID, HID], bf16, "wr2")
        wk1_t = ld(Wk1, [HID, HID * KAN_Q], bf16, "wk1")
        wk2_t = [ld(Wk2[k * P:(k + 1) * P, :], [P, HID], bf16, f"wk2{k}") for k in range(KAN_Q)]
        wc_t = ld(Wc, [HID, 1], bf16, "wc")
        brl1_t = ld(brl1, [1, HID], bf16, "brl1")
        brl2_t = ld(brl2, [1, HID], bf16, "brl2")
        bk1_t = ld(bk1, [P, KAN_Q], f32, "bk1")
        bk2_t = ld(bk2, [P, 1], f32, "bk2")
        bc_t = ld(bc, [1, 1], f32, "bc")
        id_t = ld(ident, [P, P], bf16, "id")
        inv_t = ld(inv, [P, NCH], f32, "inv")
        mg_t = ld(mg, [P, SH // 16], i16, "mg")

        ones_t = const.tile([1, P], bf16, tag="ones")
        nc.gpsimd.memset(ones_t[:], 1.0)

        def gather(g_ap, src_ap, idx_ap, ni, qn):
            """Split gather into <=SPLIT_NI single-packet sub-gathers."""
            done = 0
            while done < ni:
                sub = min(SPLIT_NI, ni - done)
                nc.gpsimd.dma_gather(
                    g_ap[:, done // P:(done + sub) // P, :], src_ap,
                    idx_ap[:, done // 16:(done + sub) // 16],
                    sub, sub, HID, single_packet=SINGLE_PACKET,
                    queue_num=qn % 4)
                done += sub
                qn += 1
            return qn

        # ---------------- sharded table GEMM -> local contribution -> AllGather
        def table_gemm(tloc, lhts_fn, nreal):
            for g0 in (0, NCHC // 2):
                nb = NCHC // 2
                bounce = work.tile([P, nb, HID], bf16, tag="bnc")
                for j in range(nb):
                    t = g0 + j
                    if t < nreal:
                        ps = psum.tile([P, HID], mybir.dt.float32, tag="mm128")
                        lhts, ws = lhts_fn(t)
                        for kk, lh in enumerate(lhts):
                            nc.tensor.matmul(ps[:], lh, ws[kk][:],
                                             start=(kk == 0), stop=(kk == len(lhts) - 1))
                        nc.scalar.activation(bounce[:, j, :], ps[:], AF.Copy)
                    else:
                        nc.vector.memset(bounce[:, j, :], 0.0)
                nc.sync.dma_start(
                    out=tloc[g0 * P:(g0 + nb) * P, :].rearrange(
                        "(t p) f -> p t f", p=P),
                    in_=bounce[:])

        # table1loc = x_shard @ W_l1 (natural order; host zero-pads tail cols)
        xts_t = const.tile([P, 2, SHC], bf16, tag="xts")
        nc.sync.dma_start(out=xts_t[:, 0, :], in_=xTs[0:P, :])
        nc.sync.dma_start(out=xts_t[:, 1, :], in_=xTs[P:2 * P, :])

        def l1_lhts(t):
            return ([xts_t[:, 0, t * P:(t + 1) * P], xts_t[:, 1, t * P:(t + 1) * P]],
                    wl1_t)
        table_gemm(table1loc, l1_lhts, NCHC)

        import concourse.mybir as mb2
        nc.gpsimd.collective_compute(
            "AllGather", mb2.AluOpType.bypass,
            replica_groups=[list(range(cfg.NCORES))],
            ins=[table1loc.opt()], outs=[table1.opt()])

        # ---------------- aggregation pass
        # Pieces of the chunk range merge + run their epilogue as soon as the
        # B-call tail stops touching them (call k covers chunks [0, nck)), so
        # downstream compute overlaps the low-degree gather tail.
        PIECES = [(39, NCH), (26, 39), (13, 26), (0, 13)]

        def agg_pass(tbl, idx_dram, epi_cb):
            idxt = gp.tile([P, SIDXP], i16, tag="idxt")
            nc.sync.dma_start(out=idxt[:, :SIDX], in_=idx_dram[:, :SIDX])
            accA = accp.tile([P, NCH, HID], f32, tag="accA")
            accB = accp.tile([P, NCH, HID], f32, tag="accB")
            nc.vector.memset(accA[:], 0.0)
            nc.vector.memset(accB[:], 0.0)
            stable = {}
            for c0, c1 in PIECES:
                k = max(i for i, nck in enumerate(callsA) if nck > c0)
                stable.setdefault(k, []).append((c0, c1))

            def merge_piece(c0, c1, pi):
                # accB is fully merged into scratchB by now; gathered rows are
                # arbitrary permB positions, so the full scratchB is required.
                ncp = c1 - c0
                gm = gp.tile([P, 13, HID], f32, tag="gm")
                gather(gm, scratchB[:], mg_t[:, c0 * 8:(c0 + ncp) * 8],
                       ncp * P, pi)
                nc.vector.tensor_add(
                    accA[:, c0:c1, :].rearrange("p c f -> p (c f)"),
                    accA[:, c0:c1, :].rearrange("p c f -> p (c f)"),
                    gm[:, :ncp, :].rearrange("p c f -> p (c f)"))

            # B half first: its accumulator round-trips through DRAM while the
            # A half gathers; merge+epilogue pieces then fire during the
            # low-degree tail of the A calls.
            qn, col, pi = 0, 0, 0
            colA0 = sum((nck * P) // 16 for nck in callsA)
            for calls, acc, base in ((callsB, accB, TH), (callsA, accA, 0)):
                src_ap = tbl[TH:V2, :] if base else tbl[0:TH, :]
                col = colA0 if base else 0
                for ci, nck in enumerate(calls):
                    ni = nck * P
                    s = ni // 16
                    big = nck > 12
                    g = gp.tile([P, NCH if big else 12, HID], bf16,
                                tag=("gb%d" if big else "gs%d") % (ci % 2))
                    qn = gather(g, src_ap, idxt[:, col:col + s], ni, qn)
                    nc.vector.tensor_add(
                        acc.rearrange("p c f -> p (c f)")[:, :nck * HID],
                        acc.rearrange("p c f -> p (c f)")[:, :nck * HID],
                        g.rearrange("p c f -> p (c f)")[:, :nck * HID])
                    col += s
                    if not base:
                        for c0, c1 in stable.get(ci, []):
                            merge_piece(c0, c1, pi)
                            epi_cb(accA, c0, c1)
                            pi += 1
                if base:
                    nc.sync.dma_start(
                        out=scratchB[:].rearrange("(c p) f -> p c f", p=P),
                        in_=accB[:])
            return accA

        # ---- layer 1
        xtpa_t = const.tile([P, 2, SH], bf16, tag="xtpa")
        nc.sync.dma_start(out=xtpa_t[:, 0, :], in_=xTpA[0:P, :])
        nc.sync.dma_start(out=xtpa_t[:, 1, :], in_=xTpA[P:2 * P, :])
        h1bnc = wrk1.tile([P, NCH, HID], bf16, tag="h1b")

        def epi1(accA1, c0, c1):
            for c in range(c0, c1):
                ps = psum.tile([P, HID], mybir.dt.float32, tag="mm128")
                nc.tensor.matmul(ps[:], xtpa_t[:, 0, c * P:(c + 1) * P],
                                 wr1_t[0][:], start=True, stop=False)
                nc.tensor.matmul(ps[:], xtpa_t[:, 1, c * P:(c + 1) * P],
                                 wr1_t[1][:], start=False, stop=False)
                nc.tensor.matmul(ps[:], ones_t[:], brl1_t[:],
                                 start=False, stop=True)
                tmp = work.tile([P, HID], f32, tag="epi")
                nc.vector.scalar_tensor_tensor(
                    tmp[:], accA1[:, c, :], inv_t[:, c:c + 1], ps[:],
                    op0=ALU.mult, op1=ALU.add)
                nc.scalar.activation(h1bnc[:, c, :], tmp[:], AF.Relu)

        agg_pass(table1, idx1, epi1)
        nc.sync.dma_start(out=h1loc[:].rearrange("(c p) f -> p c f", p=P),
                          in_=h1bnc[:])

        # h1 transposed (feature-major) for table2 GEMM + r2 root term
        h1T = wrk1.tile([P, SH], bf16, tag="h1T")
        nc.sync.dma_start_transpose(h1T[:], h1loc[:])

        # ---- table2loc = h1 @ W_l2 (permA order) + AllGather
        def l2_lhts(t):
            return ([h1T[:, t * P:(t + 1) * P]], [wl2_t])
        table_gemm(table2loc, l2_lhts, NCH)

        nc.gpsimd.collective_compute(
            "AllGather", mb2.AluOpType.bypass,
            replica_groups=[list(range(cfg.NCORES))],
            ins=[table2loc.opt()], outs=[table2.opt()])

        # ---- layer 2 + fused KAN/classifier
        def epi2(accA2, c0, c1):
            lgrow = wrk1.tile([1, 13 * P], f32, tag="lg")
            for c in range(c0, c1):
                ps = psum.tile([P, HID], mybir.dt.float32, tag="mm128")
                nc.tensor.matmul(ps[:], h1T[:, c * P:(c + 1) * P], wr2_t[:],
                                 start=True, stop=False)
                nc.tensor.matmul(ps[:], ones_t[:], brl2_t[:],
                                 start=False, stop=True)
                tmp = work.tile([P, HID], f32, tag="epi")
                nc.vector.scalar_tensor_tensor(
                    tmp[:], accA2[:, c, :], inv_t[:, c:c + 1], ps[:],
                    op0=ALU.mult, op1=ALU.add)
                h2c = work.tile([P, HID], bf16, tag="h2c")
                nc.scalar.activation(h2c[:], tmp[:], AF.Relu)
                pt = psum5.tile([P, P], bf16, tag="tr")
                nc.tensor.transpose(pt[:], h2c[:], id_t[:])
                h2Tc = work.tile([P, P], bf16, tag="h2T")
                nc.vector.tensor_copy(h2Tc[:], pt[:])
                # KAN (bf16) for this chunk
                g1 = work.tile([P, KAN_Q, P], bf16, tag="g1")
                for j in range(KAN_Q):
                    psk = psum5.tile([P, P], mybir.dt.float32, tag="kan")
                    nc.tensor.matmul(psk[:], wk1_t[:, j * P:(j + 1) * P],
                                     h2Tc[:], start=True, stop=True)
                    nc.scalar.activation(g1[:, j, :], psk[:], GELU,
                                         bias=bk1_t[:, j:j + 1])
                ps2 = psum5.tile([P, P], mybir.dt.float32, tag="kan")
                for j in range(KAN_Q):
                    nc.tensor.matmul(ps2[:], wk2_t[j][:], g1[:, j, :],
                                     start=(j == 0), stop=(j == KAN_Q - 1))
                g2 = work.tile([P, P], bf16, tag="g2")
                nc.scalar.activation(g2[:], ps2[:], GELU, bias=bk2_t[:, 0:1])
                ps3 = psum5.tile([1, P], mybir.dt.float32, tag="kan")
                nc.tensor.matmul(ps3[:], wc_t[:], g2[:], start=True, stop=True)
                nc.vector.tensor_scalar_add(
                    lgrow[0:1, (c - c0) * P:(c - c0 + 1) * P], ps3[:],
                    bc_t[0:1, 0:1])
            nc.sync.dma_start(out=logits[None, c0 * P:c1 * P],
                              in_=lgrow[0:1, :(c1 - c0) * P])

        agg_pass(table2, idx2, epi2)

    nc.compile()
    return nc


# ================================================================ entry point
_CACHE = {}


def _run(cfg, x, edge_index, W_l1, b_l1, W_r1, W_l2, b_l2, W_r2,
         W_k1, b_k1, W_k2, b_k2, W_c, b_c, gelu_func="Gelu_apprx_tanh",
         trace=False, tmpdir=None, _res_out=None):
    from concourse.bass_utils import run_bass_kernel_spmd

    x = np.asarray(x, _F32)
    cores, plans = preprocess(edge_index, cfg)
    key = (cfg.N, cfg.E, cfg.NCORES, plans["sidx"],
           tuple(plans["callsA"]), tuple(plans["callsB"]), gelu_func)
    if key not in _CACHE:
        _CACHE[key] = build(cfg, plans, gelu_func)
    nc = _CACHE[key]

    common = dict(
        Wl1=np.asarray(W_l1, _F32).astype(_BF16),
        Wr1=np.asarray(W_r1, _F32).astype(_BF16),
        Wl2=np.asarray(W_l2, _F32).astype(_BF16),
        Wr2=np.asarray(W_r2, _F32).astype(_BF16),
        Wk1=np.asarray(W_k1, _F32).astype(_BF16),
        Wk2=np.asarray(W_k2, _F32).astype(_BF16),
        Wc=np.asarray(W_c, _F32).astype(_BF16),
        brl1=np.asarray(b_l1, _F32).reshape(1, HID).astype(_BF16),
        brl2=np.asarray(b_l2, _F32).reshape(1, HID).astype(_BF16),
        bk1=np.ascontiguousarray(
            np.asarray(b_k1, _F32).reshape(KAN_Q, P).T),
        bk2=np.asarray(b_k2, _F32).reshape(P, 1),
        bc=np.asarray(b_c, _F32).reshape(1, 1),
        ident=np.eye(P, dtype=_F32).astype(_BF16),
    )

    SR, SH, SHC = cfg.SHARD_REAL, cfg.SH, cfg.SHC
    sidxp = max(plans["sidx"], 8)
    in_maps = []
    for c in range(cfg.NCORES):
        cc = cores[c]
        lo = c * SR
        xs = np.zeros((SHC, IN_DIM), _F32)
        xs[:SR] = x[lo:lo + SR]
        permA = cc["permA"]
        nodes = np.minimum(permA, SR - 1) + lo
        xp = x[nodes].copy()
        xp[permA >= SR] = 0.0
        m = dict(common)
        m.update(
            xTs=np.ascontiguousarray(xs.T).astype(_BF16),
            xTpA=np.ascontiguousarray(xp.T).astype(_BF16),
            idx1=_pad_cols(cc["idx1"], sidxp),
            idx2=_pad_cols(cc["idx2"], sidxp),
            mg=cc["mg"], inv=cc["inv"],
        )
        in_maps.append(m)

    res = run_bass_kernel_spmd(nc, in_maps, core_ids=list(range(cfg.NCORES)),
                               trace=trace, tmpdir=tmpdir)
    if _res_out is not None:
        _res_out.append(res)
    out = np.zeros(cfg.N, _F32)
    for c in range(cfg.NCORES):
        lo = c * SR
        shard = res.results[c]["logits"]
        permA = cores[c]["permA"]
        real = permA < SR
        out[lo + permA[real]] = shard[real]
    return out


def _pad_cols(a, w):
    out = np.zeros((P, w), a.dtype)
    out[:, :a.shape[1]] = a
    return out


def kernel(**inputs):
    cfg = CFG()
    return _run(cfg, **inputs)
